# revision 18
# baseline (speedup 1.0000x reference)
"""Trainium2 Bass kernel for BertSelfAttention variant (logsigmoid-fused QK attention).

Reference computation (B=2, S=2048, D=1024, H=16, dh=64):
    q = split_heads(hidden @ Wq + bq)
    k = split_heads(hidden @ Wk + bk)
    k1 = logsigmoid(q) + q + k ; k2 = logsigmoid(k1)
    scores = -(q @ k2^T) / 8 ; probs = softmax(scores) ; ctx = probs @ q

With kk := -k2 >= 0, exactly kk = ln(1 + e^{-q-k}(1 + e^{-q})), so the
elementwise chain needs only two projections (Wq and host-fused Wq+Wk):
    ev2 = exp(-(qp + bq))        = e^{-q}    # ACT, straight from qp PSUM
    eu  = exp(-(ap + bq + bk))   = e^{-q-k}  # ACT, from ap PSUM
    ut  = eu * ev2 + eu                       # DVE mul + add
    kk  = ln(1 + ut)                          # ACT Ln, batched [128,2048]
All activations (Exp + Ln) live in the natural_log_exp_and_others table,
pre-loaded once via an explicit InstLoadActFuncSet -- no mid-kernel
activation-table swaps.

Sharding: 8 cores = 2 (batch) x 4 (head groups of 4 heads / 256 cols of Wq,Wk).
Each core computes its [2048, 256] slice of the output; host reassembles.

Device layout is fully transposed so no matmul ever needs a transposed operand:
    qT, kkT [256(dout), 2048(s)]  from  ht = hidden[b].T  (host-side transpose)
    scoresT[kpos, q] = kkT_head^T @ qT_head    (row-packed head pairs, K=64)
    expT = Exp(scoresT / 8)                    (one [128,1024] ACT op per chunk)
    ctx_aug[65, q] = sum_kpos v_aug[kpos,65]^T @ expT[kpos, q]
        v_aug = [v | 1] -> row 64 accumulates the softmax denominator.
Raw transposed ctx + denominators go back to DRAM; the host divides and
re-transposes while unsharding (no device-side finalize transposes).

Schedule (v2): both halves' projections run upfront as back-to-back matmul
chains (PE stays continuously busy -> p-state ramps to 2.4GHz early).  The
projection (qp|ap) pairs live in the same rotating PSUM slots the score
tiles use later (tag sp, 2x[128,1024]); v_aug PE-transposes are batched
4-wide into one-bank tpv tiles (tags cA/cB, which become the ctx accumulators
during streams).  In the stream phase the prev-stream ctx drains ride between
score matmuls as sp-independent PE filler, so the PE keeps long busy streaks
while ACT stays saturated on the score exps.

Matmuls run in bf16; the softmax denominator path and output stay fp32.
"""

import numpy as np

B, S, D = 2, 2048, 1024
H, DH = 16, 64
NCORES = 8
HG = 4  # head-group count (tensor parallel)
CPG = (H // HG) * DH  # cols per group = 256
NDT = D // 128  # 8 din tiles
NSC = S // 512  # 4 s-chunks (projection) == 4 q-chunks (attention)
NKC = S // 128  # 16 kpos chunks

MM_DTYPE = "bf16"  # "bf16" | "f32r" | "f32"

_compiled = None
LAST_RESULT = None


def _build():
    from contextlib import ExitStack

    import concourse.bacc as bacc
    import concourse.mybir as mybir
    import concourse.tile as tile

    from concourse.alu_op_type import AluOpType as AluOp

    f32 = mybir.dt.float32
    mmdt = {
        "bf16": mybir.dt.bfloat16,
        "f32r": mybir.dt.float32r,
        "f32": mybir.dt.float32,
    }[MM_DTYPE]
    AF = mybir.ActivationFunctionType

    nc = bacc.Bacc("TRN2", target_bir_lowering=False, debug=False)
    ht = nc.dram_tensor("ht", [D, S], mmdt, kind="ExternalInput").ap()
    # host-fused weight wall: [Wq | Wq+Wk] per row, one load per tile
    wall = nc.dram_tensor("wall", [D, 2 * CPG], mmdt, kind="ExternalInput").ap()
    # smalls cols: pbq[0:2] nbq[2:4] nbqk[4:6] ones[6:22]
    smalls = nc.dram_tensor("smalls", [128, 22], f32, kind="ExternalInput").ap()
    # identity for the v transposes, in the matmul dtype (I64 stacked twice)
    idb = nc.dram_tensor("idb", [128, 64], mmdt, kind="ExternalInput").ap()
    # 65 rows per head: 64 raw ctx rows + the softmax-denominator row, so one
    # finalize needs a single DMA (no separate dens tensor / tiny transfers).
    out = nc.dram_tensor("out", [4 * 65, S], f32, kind="ExternalOutput").ap()

    # natural_log_exp_and_others: serves every Exp and Ln in the kernel with
    # a single activation-table load (pre-placed below; the compiler's
    # per-function chooser would otherwise bounce between the exp-only and
    # ln-only tables, 1283ns per swap).
    from concourse.hw_specs import get_activation_tables

    nlx_id = list(get_activation_tables(nc.m.arch)).index(
        "natural_log_exp_and_others"
    )

    with tile.TileContext(nc) as tc, ExitStack() as ctx:
        const = ctx.enter_context(tc.tile_pool(name="const", bufs=1))
        big = ctx.enter_context(tc.tile_pool(name="big", bufs=1))
        sb = ctx.enter_context(tc.tile_pool(name="sb", bufs=2))
        # One PSUM pool, exactly 8 banks:
        #   sp  bufs=2 x [128,1024] f32 = 4 banks (proj pairs, then score tiles)
        #   cA  bufs=2 x 1 bank          (tpv transposes, then ctx head rr=0)
        #   cB  bufs=2 x 1 bank          (tpv transposes, then ctx head rr=1)
        ps = ctx.enter_context(tc.tile_pool(name="ps", bufs=1, space="PSUM"))
        csp = ctx.enter_context(tc.tile_pool(name="csp", bufs=4))

        nc.scalar.add_instruction(
            mybir.InstLoadActFuncSet(act_func_set_id=nlx_id)
        )

        sm = const.tile([128, 22], f32, tag="smalls")
        nc.sync.dma_start(sm[:], smalls[:])
        pbq_t = sm[:, 0:2]
        nbq_t = sm[:, 2:4]
        nbqk_t = sm[:, 4:6]
        ones_t = sm[:, 6:22]
        idb_t = const.tile([128, 64], mmdt, tag="idb")
        nc.sync.dma_start(idb_t[:], idb[:])

        # Inputs needed first come first, interleaved across both HWDGE
        # rings (SP + ACT): per din tile j, its weight chunk plus the sc=0
        # hidden chunk; then the remaining hidden chunks sc-major so chunk
        # sc is complete across all j before chunk sc+1 lands.
        rr_ring = [nc.sync, nc.scalar]
        ring_i = 0

        def ring():
            nonlocal ring_i
            ring_i += 1
            return rr_ring[ring_i % 2]

        wqs, was, hts = [], [], []
        for j in range(NDT):
            t_ = big.tile([128, S], mmdt, tag=f"ht{j}", name=f"hts{j}")
            hts.append(t_)
        for j in range(NDT):
            w = const.tile([128, 2 * CPG], mmdt, tag=f"w{j}", name=f"walls{j}")
            # hidden chunk before its weight tile: the first chain touches
            # ht_0/w_0 first, so pairs land in consumption order.
            ring().dma_start(
                hts[j][:, 0:512], ht[j * 128 : (j + 1) * 128, 0:512]
            )
            ring().dma_start(w[:], wall[j * 128 : (j + 1) * 128, :])
            wqs.append(w[:, 0:CPG])
            was.append(w[:, CPG : 2 * CPG])
        for sc in range(1, NSC):
            ssl = slice(sc * 512, (sc + 1) * 512)
            for j in range(NDT):
                ring().dma_start(hts[j][:, ssl], ht[j * 128 : (j + 1) * 128, ssl])

        q_sb = [big.tile([128, S], mmdt, tag=f"q{t}", name=f"q{t}") for t in range(2)]
        kk_sb = [big.tile([128, S], mmdt, tag=f"kk{t}", name=f"kk{t}") for t in range(2)]
        ut = [big.tile([128, S], f32, tag=f"ut{t}", name=f"ut{t}") for t in range(2)]
        vaug = [
            big.tile([128, NKC * 65], mmdt, tag=f"v{h}", name=f"v{h}") for h in range(4)
        ]
        # three stream-sized exp buffers; drains index [128,1024] sub-slices.
        # Three (not two) so stream i+1's exps land in a different buffer
        # than the one stream i's drains are still reading (i+1 vs i-1).
        etb = [
            big.tile([128, NKC * 1024], mmdt, tag=f"et{r}", name=f"et{r}")
            for r in range(3)
        ]

        # ---------- emission helpers ----------
        def emit_vaug_ones(t):
            for rr in range(2):
                vv = vaug[2 * t + rr][:].rearrange("p (c w) -> p c w", w=65)
                nc.vector.tensor_copy(
                    vv[:, :, 64:65], ones_t.rearrange("p (c w) -> p c w", w=1)
                )

        def emit_proj_chunk(t, sc):
            """Two matmul chains for one [dout-half, 512] chunk + elementwise."""
            ssl = slice(sc * 512, (sc + 1) * 512)
            pr = ps.tile([128, 1024], f32, tag="sp", name="pr", bufs=2)
            for j in range(NDT):
                nc.tensor.matmul(
                    pr[:, 0:512],
                    lhsT=wqs[j][:, t * 128 : (t + 1) * 128],
                    rhs=hts[j][:, ssl],
                    start=(j == 0),
                    stop=(j == NDT - 1),
                )
            for j in range(NDT):
                nc.tensor.matmul(
                    pr[:, 512:1024],
                    lhsT=was[j][:, t * 128 : (t + 1) * 128],
                    rhs=hts[j][:, ssl],
                    start=(j == 0),
                    stop=(j == NDT - 1),
                )
            ev2 = sb.tile([128, 512], f32, tag="ev2")
            nc.scalar.activation(
                ev2[:], pr[:, 0:512], AF.Exp, bias=nbq_t[:, t : t + 1], scale=-1.0
            )
            eu = sb.tile([128, 512], f32, tag="eu")
            nc.scalar.activation(
                eu[:], pr[:, 512:1024], AF.Exp, bias=nbqk_t[:, t : t + 1], scale=-1.0
            )
            nc.vector.tensor_scalar_add(q_sb[t][:, ssl], pr[:, 0:512], pbq_t[:, t : t + 1])
            # ut = (ev2 + 1) * eu  ==  e^{-q-k}(1 + e^{-q})  in one DVE op
            nc.vector.scalar_tensor_tensor(
                ut[t][:, ssl], ev2[:], 1.0, eu[:], AluOp.add, AluOp.mult
            )

        def emit_kk(t):
            nc.scalar.activation(kk_sb[t][:], ut[t][:], AF.Ln, bias=1.0, scale=1.0)

        def emit_vaug_chunk(t, sc, tag):
            """PE-transpose one [64,512] q chunk per head of half t, batched:
            4 [128,64] transposes into one tpv tile, one DVE copy out."""
            for rr in range(2):
                lh = 2 * t + rr
                hsl = slice(rr * 64, rr * 64 + 64)
                tpv = ps.tile(
                    [128, 256], mmdt, tag=tag[rr], name="tpv", bufs=2
                )
                for jj in range(4):
                    j = 4 * sc + jj
                    nc.tensor.transpose(
                        tpv[:, jj * 64 : (jj + 1) * 64],
                        q_sb[t][hsl, j * 128 : (j + 1) * 128],
                        idb_t[hsl, 0:64],
                    )
                vv = vaug[lh][:].rearrange("p (c w) -> p c w", w=65)
                nc.vector.tensor_copy(
                    vv[:, 4 * sc : 4 * sc + 4, 0:64],
                    tpv[:].rearrange("p (c w) -> p c w", w=64),
                )

        def emit_drain_chunk(prev_state, kc_rev, immediate=False):
            qc_p, t_p, ebuf_p, ctxs_p = prev_state
            for rr in range(2):
                nc.tensor.matmul(
                    ctxs_p[rr][:],
                    lhsT=vaug[2 * t_p + rr][:, kc_rev * 65 : kc_rev * 65 + 65],
                    rhs=ebuf_p[:, kc_rev * 1024 + rr * 512 : kc_rev * 1024 + rr * 512 + 512],
                    start=(kc_rev == (0 if immediate else NKC - 1)),
                    stop=(kc_rev == (NKC - 1 if immediate else 0)),
                )

        def emit_finalize(prev_state):
            qc_p, t_p, ebuf_p, ctxs_p = prev_state
            qsl_p = slice(qc_p * 512, (qc_p + 1) * 512)
            for rr in range(2):
                lh = 2 * t_p + rr
                cs = csp.tile([128, 512], f32, tag="cs")
                nc.vector.tensor_copy(cs[0:65, :], ctxs_p[rr][:])
                # one 65-row DMA (ctx + denominator); alternate dispatch
                # queues so tail finalizes don't serialize on one ring
                eng = nc.sync if rr == 0 else nc.gpsimd
                eng.dma_start(out[lh * 65 : lh * 65 + 65, qsl_p], cs[0:65, :])

        # ---------- schedule ----------
        # Phase P: all projections.  sc-major so each freshly-DMA'd hidden
        # chunk feeds two chain pairs back-to-back (PE stays ahead of the
        # input stream).  The v_aug transposes for a chunk are emitted one
        # iteration later, so the PE never waits on the DVE q-add that
        # produces their input.
        emit_vaug_ones(0)
        emit_vaug_ones(1)
        order = [(sc, t) for sc in range(NSC) for t in range(2)]
        for it, (sc, t) in enumerate(order):
            emit_proj_chunk(t, sc)
            if it >= 1:
                psc, pt = order[it - 1]
                emit_vaug_chunk(pt, psc, tag=("cA", "cB"))
        emit_kk(0)
        emit_vaug_chunk(order[-1][1], order[-1][0], tag=("cA", "cB"))
        emit_kk(1)

        # Phase S: one long pipeline of 8 t-major streams.  While ACT streams
        # the exps of stream i, the PE drains stream i-1's ctx accumulation
        # (descending kc so only the first drain matmul carries a semaphore
        # wait).
        streams = [(qc, t) for t in range(2) for qc in range(NSC)]
        prev = None
        for i, (qc, t) in enumerate(streams):
            qsl = slice(qc * 512, (qc + 1) * 512)
            last = i == len(streams) - 1
            ebuf = etb[i % 3]
            ctxs_now = None
            if last:
                # final stream: drain immediately per chunk (ascending kc), so
                # only the finalize remains after the pipeline.
                ctxs_now = [
                    ps.tile([65, 512], f32, tag="cA", name="ctxA", bufs=2),
                    ps.tile([65, 512], f32, tag="cB", name="ctxB", bufs=2),
                ]
            # Drains ride 3 kc behind the scores: drain index kc-DD touches
            # et chunks whose exps ACT has certainly retired, so the in-order
            # PE queue never blocks on a lagging exp at stream boundaries.
            DD = 3
            for kc in range(NKC):
                ksl = slice(kc * 128, (kc + 1) * 128)
                sp = ps.tile([128, 1024], f32, tag="sp", name="sp", bufs=2)
                nc.tensor.matmul(
                    sp[:, 0:512],
                    lhsT=kk_sb[t][0:64, ksl],
                    rhs=q_sb[t][0:64, qsl],
                    start=True,
                    stop=True,
                )
                nc.tensor.matmul(
                    sp[:, 512:1024],
                    lhsT=kk_sb[t][64:128, ksl],
                    rhs=q_sb[t][64:128, qsl],
                    start=True,
                    stop=True,
                )
                nc.scalar.activation(
                    ebuf[:, kc * 1024 : (kc + 1) * 1024], sp[:], AF.Exp, scale=0.125
                )
                if kc >= DD:
                    if prev is not None:
                        emit_drain_chunk(prev, NKC - 1 - (kc - DD))
                    if last:
                        emit_drain_chunk((qc, t, ebuf, ctxs_now), kc - DD, immediate=True)
            for kd in range(DD):
                if prev is not None:
                    emit_drain_chunk(prev, DD - 1 - kd)
                if last:
                    emit_drain_chunk((qc, t, ebuf, ctxs_now), NKC - DD + kd, immediate=True)
            if prev is not None:
                emit_finalize(prev)
            if last:
                emit_finalize((qc, t, ebuf, ctxs_now))
                prev = None
            else:
                ctxs = [
                    ps.tile([65, 512], f32, tag="cA", name="ctxA", bufs=2),
                    ps.tile([65, 512], f32, tag="cB", name="ctxB", bufs=2),
                ]
                prev = (qc, t, ebuf, ctxs)

    nc.compile()
    return nc


def kernel(hidden_states, attention_mask, Wq, bq, Wk, bk):
    global _compiled, LAST_RESULT
    hs = np.asarray(hidden_states, dtype=np.float32)
    am = np.asarray(attention_mask)
    Wq = np.asarray(Wq, dtype=np.float32)
    Wk = np.asarray(Wk, dtype=np.float32)
    bq = np.asarray(bq, dtype=np.float32)
    bk = np.asarray(bk, dtype=np.float32)

    if _compiled is None:
        _compiled = _build()
    nc = _compiled

    from concourse.bass_utils import run_bass_kernel_spmd

    if MM_DTYPE == "bf16":
        import ml_dtypes

        def to_mmdt(x):
            return np.ascontiguousarray(
                np.asarray(x, np.float32).astype(ml_dtypes.bfloat16)
            )

    elif MM_DTYPE == "f32r":

        def to_mmdt(x):
            # fp32r = 1s/8e/11m (top 20 bits of fp32), round-to-nearest-even
            b = np.ascontiguousarray(x, dtype=np.float32).view(np.uint32)
            lsb = (b >> np.uint32(12)) & np.uint32(1)
            r = (b + np.uint32(0x7FF) + lsb) & np.uint32(0xFFFFF000)
            return r.view(np.float32)

    else:

        def to_mmdt(x):
            return np.ascontiguousarray(x, dtype=np.float32)

    idb = to_mmdt(np.tile(np.eye(64, dtype=np.float32), (2, 1)))
    in_maps = []
    for c in range(NCORES):
        b, g = c // HG, c % HG
        cols = slice(g * CPG, (g + 1) * CPG)
        bq_s = bq[cols].reshape(2, 128).T
        bk_s = bk[cols].reshape(2, 128).T
        smalls = np.concatenate(
            [bq_s, -bq_s, -(bq_s + bk_s), np.ones((128, 16), np.float32)],
            axis=1,
        ).astype(np.float32)
        in_maps.append(
            {
                "ht": to_mmdt(hs[b].T),
                "wall": to_mmdt(
                    np.concatenate(
                        [Wq[:, cols], Wq[:, cols] + Wk[:, cols]],
                        axis=1,
                    )
                ),
                "smalls": np.ascontiguousarray(smalls),
                "idb": idb,
            }
        )

    res = run_bass_kernel_spmd(nc, in_maps, list(range(NCORES)))
    LAST_RESULT = res

    outp = np.empty((B, S, H * DH), dtype=np.float32)
    for c in range(NCORES):
        b, g = c // HG, c % HG
        raw = res.results[c]["out"].reshape(4, 65, S)  # per head: 64 ctx + den
        ctxT = raw[:, 0:64, :] / raw[:, 64:65, :]
        outp[b, :, g * CPG : (g + 1) * CPG] = ctxT.reshape(CPG, S).T

    # attention_mask==0 masks whole query rows -> uniform probs -> ctx row is
    # the mean of q over all key positions. Never triggers for all-ones masks.
    if (am == 0).any():
        for b in range(B):
            rows = np.nonzero(am[b] == 0)[0]
            if rows.size:
                q_full = hs[b] @ Wq + bq
                outp[b, rows, :] = q_full.mean(axis=0)
    return outp
